# revision 1
# baseline (speedup 1.0000x reference)
"""HGT link predictor on 8 Trainium2 NeuronCores (Bass/Tile SPMD kernel).

Strategy (hardcoded for nn_HGTLinkPredictor, N=50000 E=800000 P=100000 C=128 H=4 D=32):
 - Shard dst nodes (and their incoming edges) across 8 cores in contiguous
   128-node blocks. Edges sorted by dst on host.
 - Per layer: each core computes q/k/v projections for its node shard
   (relation transforms + attention scaling folded into the weights on host),
   AllGathers k/v so every core can gather k[src], v[src] rows by indirect DMA.
 - Segment softmax/weighted-sum over incoming edges of each dst node are done
   per 128-node block with one-hot selection matrices multiplied on the PE
   into PSUM accumulators (denominator division is algebraically deferred to
   the block epilogue, so a single pass over edges suffices).
 - Link decode is data-parallel over candidate edges with indirect gathers of
   the final node embeddings (AllGathered once).
"""

import math
import numpy as np
from contextlib import ExitStack

import concourse.bass as bass
import concourse.tile as tile
from concourse import bacc, mybir
from concourse import bass_utils
from concourse.masks import make_identity

F32 = mybir.dt.float32
I32 = mybir.dt.int32
AF = mybir.ActivationFunctionType
OP = mybir.AluOpType

CORES = 8
EPS = 1e-30


def _expand_last(ap, n):
    """Append a step-0 (broadcast) innermost dim of size n to an AP."""
    new = [list(p) for p in ap.ap] + [[0, n]]
    return bass.AP(ap.tensor, ap.offset, new)


# ----------------------------------------------------------------- host prep

def _host_prep(x, edge_index, pos_edge_index, neg_edge_index):
    N, C = x.shape
    E = edge_index.shape[1]
    P = pos_edge_index.shape[1]

    NPC = int(math.ceil(N / (CORES * 128))) * 128   # nodes per core (padded)
    BPC = NPC // 128                                # blocks per core
    NPAD = NPC * CORES

    src = edge_index[0].astype(np.int64)
    dst = edge_index[1].astype(np.int64)
    order = np.argsort(dst, kind="stable")
    s_src, s_dst = src[order], dst[order]

    core_of = s_dst // NPC
    blk_of = (s_dst % NPC) // 128
    gblk = core_of * BPC + blk_of          # global block id 0..CORES*BPC-1

    # per (core, block) counts -> uniform tile counts per block index
    cnt = np.zeros((CORES, BPC), dtype=np.int64)
    np.add.at(cnt, (core_of, blk_of), 1)
    T_b = np.maximum(1, np.ceil(cnt.max(axis=0) / 128).astype(np.int64))  # [BPC]
    tiles_total = int(T_b.sum())

    # slot in edge arrays for each sorted edge: per (core, block) sequential
    blk_starts = np.concatenate([[0], np.cumsum(T_b)])[:-1] * 128  # per block idx within core
    # position of edge within its (core, block) group
    # edges are sorted by dst hence grouped by gblk already
    grp_start = np.zeros(CORES * BPC + 1, dtype=np.int64)
    np.add.at(grp_start, gblk + 1, 1)
    grp_start = np.cumsum(grp_start)
    pos_in_grp = np.arange(E) - grp_start[gblk]

    cap = tiles_total * 128
    ekv = np.zeros((CORES, cap), dtype=np.int32)     # gather row in kv_full
    eqr = np.zeros((CORES, cap), dtype=np.int32)     # gather row in q_dram (local)
    eslot = np.full((CORES, cap), -1.0, dtype=np.float32)  # -1 => padding edge

    flat_pos = blk_starts[blk_of] + pos_in_grp       # position within core's edge array
    r = s_src // NPC
    i = s_src % NPC
    kv_row = r * (2 * NPC) + i
    np_c = core_of.astype(np.int64)
    ekv[np_c, flat_pos] = kv_row.astype(np.int32)
    eqr[np_c, flat_pos] = (s_dst - np_c * NPC).astype(np.int32)
    eslot[np_c, flat_pos] = (s_dst % 128).astype(np.float32)

    # reshape to [128, tiles_total] partition-major: entry [p, t] = edge t*128+p
    ekv = ekv.reshape(CORES, tiles_total, 128).transpose(0, 2, 1).copy()
    eqr = eqr.reshape(CORES, tiles_total, 128).transpose(0, 2, 1).copy()
    eslot = eslot.reshape(CORES, tiles_total, 128).transpose(0, 2, 1).copy()

    # decode shards
    PC = int(math.ceil(P / CORES))
    DT = int(math.ceil(PC / 128))
    PPC = DT * 128
    dec = np.zeros((CORES, 128, 4 * DT), dtype=np.int32)
    valid = []
    for c in range(CORES):
        lo = min(c * PC, P)
        hi = min(lo + PC, P)
        valid.append(hi - lo)
        for g, arr in enumerate((pos_edge_index[0], pos_edge_index[1],
                                 neg_edge_index[0], neg_edge_index[1])):
            a = np.zeros(PPC, dtype=np.int32)
            a[: hi - lo] = arr[lo:hi]
            dec[c, :, g * DT:(g + 1) * DT] = a.reshape(DT, 128).T

    # x shards (zero-padded)
    xs = np.zeros((CORES, NPC, C), dtype=np.float32)
    xpad = np.zeros((NPAD, C), dtype=np.float32)
    xpad[:N] = x
    for c in range(CORES):
        xs[c] = xpad[c * NPC:(c + 1) * NPC]

    meta = dict(N=N, C=C, E=E, P=P, NPC=NPC, BPC=BPC, NPAD=NPAD,
                T_b=tuple(int(t) for t in T_b), tiles_total=tiles_total,
                DT=DT, PC=PC, valid=valid)
    arrays = dict(ekv=ekv, eqr=eqr, eslot=eslot, dec=dec, xs=xs)
    return meta, arrays


def _prep_weights(inputs, H, D):
    """Fold relation transforms + attention scale into the linear weights."""
    C = inputs["W1k"].shape[0]
    out = {}
    for l in (1, 2):
        a_rel = np.asarray(inputs[f"a{l}"], np.float64)   # [H,D,D] (k transform)
        m_rel = np.asarray(inputs[f"m{l}"], np.float64)   # [H,D,D] (v transform)
        p_rel = np.asarray(inputs[f"p{l}"], np.float64)   # [H]
        A = np.zeros((C, C)); M = np.zeros((C, C))
        for h in range(H):
            A[h * D:(h + 1) * D, h * D:(h + 1) * D] = a_rel[h]
            M[h * D:(h + 1) * D, h * D:(h + 1) * D] = m_rel[h]
        qscale = np.repeat(p_rel / np.sqrt(D), D)         # [C]
        Wq = np.asarray(inputs[f"W{l}q"], np.float64) * qscale
        bq = np.asarray(inputs[f"b{l}q"], np.float64) * qscale
        Wk = np.asarray(inputs[f"W{l}k"], np.float64) @ A
        bk = np.asarray(inputs[f"b{l}k"], np.float64) @ A
        Wv = np.asarray(inputs[f"W{l}v"], np.float64) @ M
        bv = np.asarray(inputs[f"b{l}v"], np.float64) @ M
        a_sig = float(1.0 / (1.0 + np.exp(-float(inputs[f"skip{l}"]))))
        out[f"Wq{l}"] = Wq.astype(np.float32)
        out[f"Wk{l}"] = Wk.astype(np.float32)
        out[f"Wv{l}"] = Wv.astype(np.float32)
        out[f"Wo{l}"] = np.asarray(inputs[f"Wo{l}"], np.float32)
        out[f"bq{l}"] = np.broadcast_to(bq.astype(np.float32), (128, C)).copy()
        out[f"bk{l}"] = np.broadcast_to(bk.astype(np.float32), (128, C)).copy()
        out[f"bv{l}"] = np.broadcast_to(bv.astype(np.float32), (128, C)).copy()
        out[f"boa{l}"] = np.broadcast_to(
            (a_sig * np.asarray(inputs[f"bo{l}"], np.float64)).astype(np.float32),
            (128, C)).copy()
        out[f"asig{l}"] = a_sig
    Wlp = np.asarray(inputs["Wlp"], np.float32)
    out["w1"] = np.broadcast_to(Wlp[:C, 0], (128, C)).copy()
    out["w2"] = np.broadcast_to(Wlp[C:, 0], (128, C)).copy()
    out["blp"] = float(np.asarray(inputs["blp"]).reshape(-1)[0])
    return out


# ------------------------------------------------------------------- program

def _build_program(meta, asig1, asig2, blp, gelu_mode="hw"):
    NPC, BPC, NPAD = meta["NPC"], meta["BPC"], meta["NPAD"]
    T_b, tiles_total, DT = meta["T_b"], meta["tiles_total"], meta["DT"]
    Tmax = max(T_b)
    C = meta["C"]

    nc = bacc.Bacc("TRN2", target_bir_lowering=False, debug=False,
                   num_devices=CORES)

    # --- I/O -------------------------------------------------------------
    x_in = nc.dram_tensor("x_shard", [NPC, C], F32, kind="ExternalInput").ap()
    ekv_in = nc.dram_tensor("ekv", [128, tiles_total], I32, kind="ExternalInput").ap()
    eqr_in = nc.dram_tensor("eqr", [128, tiles_total], I32, kind="ExternalInput").ap()
    eslot_in = nc.dram_tensor("eslot", [128, tiles_total], F32, kind="ExternalInput").ap()
    wnames = []
    for l in (1, 2):
        wnames += [f"Wq{l}", f"Wk{l}", f"Wv{l}", f"Wo{l}",
                   f"bq{l}", f"bk{l}", f"bv{l}", f"boa{l}"]
    wnames += ["w1", "w2"]
    w_in = {n: nc.dram_tensor(n, [128, C], F32, kind="ExternalInput").ap()
            for n in wnames}
    uv_out = nc.dram_tensor("uv_out", [NPC, 2], F32, kind="ExternalOutput").ap()

    with tile.TileContext(nc) as tc, ExitStack() as ctx:
        sb = ctx.enter_context(tc.tile_pool(name="sb", bufs=2))
        cpool = ctx.enter_context(tc.tile_pool(name="const", bufs=1))
        psum = ctx.enter_context(tc.tile_pool(name="ps", bufs=2, space="PSUM"))
        dram = ctx.enter_context(tc.tile_pool(name="dr", bufs=1, space="DRAM"))

        # --- constants into SBUF ----------------------------------------
        W = {}
        for n in wnames:
            W[n] = cpool.tile([128, C], F32, tag=f"w_{n}", name=f"wt_{n}")
            nc.sync.dma_start(W[n][:], w_in[n][:])
        ekv_sb = cpool.tile([128, tiles_total], I32, tag="ekv")
        nc.sync.dma_start(ekv_sb[:], ekv_in[:])
        eqr_sb = cpool.tile([128, tiles_total], I32, tag="eqr")
        nc.sync.dma_start(eqr_sb[:], eqr_in[:])
        eslot_sb = cpool.tile([128, tiles_total], F32, tag="eslot")
        nc.sync.dma_start(eslot_sb[:], eslot_in[:])

        ident = cpool.tile([128, 128], F32, tag="ident")
        make_identity(nc, ident[:])
        iota_i = cpool.tile([128, Tmax * 128], I32, tag="iota_i")
        nc.gpsimd.iota(iota_i[:], pattern=[[0, Tmax], [1, 128]], base=0,
                       channel_multiplier=0)
        iota_f = cpool.tile([128, Tmax * 128], F32, tag="iota_f")
        nc.vector.tensor_copy(iota_f[:], iota_i[:])

        # --- DRAM scratch ------------------------------------------------
        q_dram = [dram.tile([NPC, C], F32, tag=f"q{l}", name=f"q_dram{l}") for l in (0, 1)]
        kv_shard = [dram.tile([2 * NPC, C], F32, tag=f"kvs{l}", name=f"kv_shard{l}") for l in (0, 1)]
        kv_full = [dram.tile([CORES * 2 * NPC, C], F32, tag=f"kvf{l}", name=f"kv_full{l}") for l in (0, 1)]
        h1_dram = dram.tile([NPC, C], F32, tag="h1")
        z_shard = dram.tile([NPC, C], F32, tag="zs")

        def layer(li, src_feat, h_out, asig):
            l = li + 1
            qd, kvs, kvf = q_dram[li], kv_shard[li], kv_full[li]
            # ---- projections for own shard ----
            for i in range(BPC):
                f = sb.tile([128, C], F32, tag="fblk")
                nc.sync.dma_start(f[:], src_feat[i * 128:(i + 1) * 128, :])
                fT_ps = psum.tile([128, 128], F32, tag="tr")
                nc.tensor.transpose(out=fT_ps[:], in_=f[:], identity=ident[:])
                fT = sb.tile([128, 128], F32, tag="fT")
                nc.vector.tensor_copy(fT[:], fT_ps[:])
                for wn, bn, dst, roff in ((f"Wq{l}", f"bq{l}", qd, 0),
                                          (f"Wk{l}", f"bk{l}", kvs, 0),
                                          (f"Wv{l}", f"bv{l}", kvs, NPC)):
                    ps = psum.tile([128, 128], F32, tag="mm")
                    nc.tensor.matmul(out=ps[:], lhsT=fT[:], rhs=W[wn][:],
                                     start=True, stop=True)
                    o = sb.tile([128, C], F32, tag="proj_o")
                    nc.vector.tensor_tensor(out=o[:], in0=ps[:], in1=W[bn][:],
                                            op=OP.add)
                    nc.sync.dma_start(
                        dst[roff + i * 128: roff + (i + 1) * 128, :], o[:])
            # ---- exchange k/v ----
            nc.gpsimd.collective_compute(
                "AllGather", OP.bypass,
                replica_groups=[list(range(CORES))],
                ins=[kvs[:]], outs=[kvf[:]])
            # ---- edge phase ----
            col = 0
            for b in range(BPC):
                T = T_b[b]
                Wd = T * 128
                kg = sb.tile([128, Tmax * 128], F32, tag="kg")
                vg = sb.tile([128, Tmax * 128], F32, tag="vg")
                qg = sb.tile([128, Tmax * 128], F32, tag="qg")
                for t in range(T):
                    sl = slice(t * 128, (t + 1) * 128)
                    nc.gpsimd.indirect_dma_start(
                        out=kg[:, sl], out_offset=None, in_=kvf[:],
                        in_offset=bass.IndirectOffsetOnAxis(
                            ap=ekv_sb[:, col + t:col + t + 1], axis=0))
                    nc.gpsimd.indirect_dma_start(
                        out=vg[:, sl], out_offset=None, in_=kvf[:],
                        in_offset=bass.IndirectOffsetOnAxis(
                            ap=ekv_sb[:, col + t:col + t + 1], axis=0),
                        element_offset=NPC * C)
                    nc.gpsimd.indirect_dma_start(
                        out=qg[:, sl], out_offset=None, in_=qd[:],
                        in_offset=bass.IndirectOffsetOnAxis(
                            ap=eqr_sb[:, col + t:col + t + 1], axis=0))
                S = sb.tile([128, Tmax * 128], F32, tag="S")
                nc.vector.tensor_tensor(
                    out=S[:, :Wd].rearrange("p (t j) -> p t j", j=128),
                    in0=iota_f[:, :Wd].rearrange("p (t j) -> p t j", j=128),
                    in1=_expand_last(eslot_sb[:, col:col + T], 128),
                    op=OP.is_equal)
                # qk dot per head
                nc.vector.tensor_tensor(out=kg[:, :Wd], in0=kg[:, :Wd],
                                        in1=qg[:, :Wd], op=OP.mult)
                alpha = sb.tile([128, Tmax * 4], F32, tag="alpha")
                nc.vector.tensor_reduce(
                    out=alpha[:, :T * 4],
                    in_=kg[:, :Wd].rearrange("p (x d) -> p x d", d=32),
                    axis=mybir.AxisListType.X, op=OP.add)
                ex = sb.tile([128, Tmax * 4], F32, tag="ex")
                nc.scalar.activation(ex[:, :T * 4], alpha[:, :T * 4], AF.Exp)
                # u = v * ex (broadcast over D)
                nc.vector.tensor_tensor(
                    out=vg[:, :Wd].rearrange("p (x d) -> p x d", d=32),
                    in0=vg[:, :Wd].rearrange("p (x d) -> p x d", d=32),
                    in1=_expand_last(ex[:, :T * 4], 32), op=OP.mult)
                den_ps = psum.tile([128, 4], F32, tag="den")
                for t in range(T):
                    nc.tensor.matmul(out=den_ps[:],
                                     lhsT=S[:, t * 128:(t + 1) * 128],
                                     rhs=ex[:, t * 4:(t + 1) * 4],
                                     start=(t == 0), stop=(t == T - 1))
                agg_ps = psum.tile([128, 128], F32, tag="agg")
                for t in range(T):
                    nc.tensor.matmul(out=agg_ps[:],
                                     lhsT=S[:, t * 128:(t + 1) * 128],
                                     rhs=vg[:, t * 128:(t + 1) * 128],
                                     start=(t == 0), stop=(t == T - 1))
                # ---- block epilogue ----
                rd = sb.tile([128, 4], F32, tag="rd")
                den_s = sb.tile([128, 4], F32, tag="den_s")
                nc.vector.tensor_scalar_add(den_s[:], den_ps[:], EPS)
                nc.vector.reciprocal(rd[:], den_s[:])
                aggn = sb.tile([128, 128], F32, tag="aggn")
                nc.vector.tensor_tensor(
                    out=aggn[:].rearrange("p (h d) -> p h d", d=32),
                    in0=agg_ps[:].rearrange("p (h d) -> p h d", d=32),
                    in1=_expand_last(rd[:], 32), op=OP.mult)
                g = sb.tile([128, 128], F32, tag="g")
                if gelu_mode == "hw":
                    nc.scalar.activation(g[:], aggn[:], AF.Gelu)
                else:
                    # sim-only tanh-approx gelu (CoreSim lacks Gelu/Erf)
                    t1 = sb.tile([128, 128], F32, tag="gel1")
                    nc.scalar.activation(t1[:], aggn[:], AF.Square)
                    nc.vector.tensor_tensor(out=t1[:], in0=t1[:], in1=aggn[:], op=OP.mult)
                    nc.vector.tensor_scalar_mul(t1[:], t1[:], 0.044715)
                    nc.vector.tensor_tensor(out=t1[:], in0=t1[:], in1=aggn[:], op=OP.add)
                    nc.scalar.activation(t1[:], t1[:], AF.Tanh, scale=0.7978845608028654)
                    nc.vector.tensor_scalar_add(t1[:], t1[:], 1.0)
                    nc.vector.tensor_tensor(out=t1[:], in0=t1[:], in1=aggn[:], op=OP.mult)
                    nc.vector.tensor_scalar_mul(g[:], t1[:], 0.5)
                gT_ps = psum.tile([128, 128], F32, tag="tr")
                nc.tensor.transpose(out=gT_ps[:], in_=g[:], identity=ident[:])
                gT = sb.tile([128, 128], F32, tag="gT")
                nc.vector.tensor_copy(gT[:], gT_ps[:])
                h_ps = psum.tile([128, 128], F32, tag="mm")
                nc.tensor.matmul(out=h_ps[:], lhsT=gT[:], rhs=W[f"Wo{l}"][:],
                                 start=True, stop=True)
                f2 = sb.tile([128, C], F32, tag="fblk2")
                nc.sync.dma_start(f2[:], src_feat[b * 128:(b + 1) * 128, :])
                hm = sb.tile([128, C], F32, tag="hm")
                nc.vector.tensor_scalar_mul(hm[:], h_ps[:], asig)
                nc.vector.tensor_tensor(out=hm[:], in0=hm[:], in1=W[f"boa{l}"][:],
                                        op=OP.add)
                xs_t = sb.tile([128, C], F32, tag="xs")
                nc.vector.tensor_scalar_mul(xs_t[:], f2[:], 1.0 - asig)
                nc.vector.tensor_tensor(out=hm[:], in0=hm[:], in1=xs_t[:],
                                        op=OP.add)
                nc.sync.dma_start(h_out[b * 128:(b + 1) * 128, :], hm[:])
                if l == 2:
                    pr = sb.tile([128, C], F32, tag="pr")
                    uv = sb.tile([128, 2], F32, tag="uv")
                    nc.vector.tensor_tensor(out=pr[:], in0=hm[:],
                                            in1=W["w1"][:], op=OP.mult)
                    nc.vector.tensor_reduce(out=uv[:, 0:1], in_=pr[:],
                                            axis=mybir.AxisListType.X, op=OP.add)
                    nc.vector.tensor_tensor(out=pr[:], in0=hm[:],
                                            in1=W["w2"][:], op=OP.mult)
                    nc.vector.tensor_reduce(out=uv[:, 1:2], in_=pr[:],
                                            axis=mybir.AxisListType.X, op=OP.add)
                    nc.sync.dma_start(uv_out[b * 128:(b + 1) * 128, :], uv[:])
                col += T

        layer(0, x_in, h1_dram[:], asig1)
        layer(1, h1_dram[:], z_shard[:], asig2)

    nc.compile()
    return nc


_CACHE = {}


def _get_program(meta, asig1, asig2, blp):
    key = (meta["N"], meta["E"], meta["P"], meta["T_b"], asig1, asig2, blp)
    if key not in _CACHE:
        _CACHE[key] = _build_program(meta, asig1, asig2, blp)
    return _CACHE[key]


def make_in_maps(inputs):
    inputs = {k: np.asarray(v) for k, v in inputs.items()}
    H, D = inputs["a1"].shape[0], inputs["a1"].shape[1]
    meta, arrays = _host_prep(inputs["x"].astype(np.float32),
                              inputs["edge_index"],
                              inputs["pos_edge_index"],
                              inputs["neg_edge_index"])
    w = _prep_weights(inputs, H, D)
    in_maps = []
    for c in range(CORES):
        m = dict(x_shard=arrays["xs"][c], ekv=arrays["ekv"][c],
                 eqr=arrays["eqr"][c], eslot=arrays["eslot"][c])
        for l in (1, 2):
            for n in (f"Wq{l}", f"Wk{l}", f"Wv{l}", f"Wo{l}",
                      f"bq{l}", f"bk{l}", f"bv{l}", f"boa{l}"):
                m[n] = w[n]
        m["w1"] = w["w1"]
        m["w2"] = w["w2"]
        in_maps.append(m)
    return meta, w, in_maps


def assemble(meta, results, inputs, blp):
    uv = np.concatenate([results[c]["uv_out"] for c in range(CORES)], axis=0)
    u1, u2 = uv[:, 0], uv[:, 1]
    pe, ne = inputs["pos_edge_index"], inputs["neg_edge_index"]
    pos = u1[pe[0]] + u2[pe[1]] + np.float32(blp)
    neg = u1[ne[0]] + u2[ne[1]] + np.float32(blp)
    return pos.astype(np.float32), neg.astype(np.float32)


def kernel(**inputs):
    meta, w, in_maps = make_in_maps(inputs)
    nc = _get_program(meta, w["asig1"], w["asig2"], w["blp"])
    res = bass_utils.run_bass_kernel_spmd(nc, in_maps,
                                          core_ids=list(range(CORES)))
    return assemble(meta, res.results, inputs, w["blp"])



# revision 7
# speedup vs baseline: 1.8388x; 1.8388x over previous
"""HGT link predictor on 8 Trainium2 NeuronCores (Bass/Tile SPMD kernel).

Strategy (hardcoded for nn_HGTLinkPredictor, N=50000 E=800000 P=100000 C=128 H=4 D=32):
 - Shard dst nodes (and their incoming edges) across 8 cores in contiguous
   128-node blocks. Edges sorted by dst on host.
 - Per layer: each core computes q/k/v projections for its node shard from a
   host-transposed feature tile (relation transforms, attention scale and the
   sigmoid-skip coefficients are all folded into the weights on host), writes
   packed [k|v] fp16 rows, AllGathers them so every core can fetch k/v of any
   src node with a single 512B-descriptor indirect DMA per edge (one packed
   row instead of separate k/v/q fetches).
 - q[dst] rows are fetched with batched Q7 dma_gather instructions whose
   indices are block-relative (dst%128) — the gather ucode routes indices
   through an fp16 conversion, so only small indices are exact.
 - Attention logits via fp16 elementwise mult + segmented reduce; exp on the
   scalar engine into a fused [ex | v*ex] bf16 tile so a single 132-column
   matmul per 128-edge tile accumulates both the softmax denominator and the
   weighted value sum in PSUM.
 - Gelu + output projection run as a deferred second pass (one activation
   table load), producing transposed features directly so layer-2 projections
   need no PE transposes. Link decode is one tiny matmul per block; final
   per-edge logits are assembled host-side from per-node partial sums.
"""

import math
import numpy as np
import ml_dtypes
from contextlib import ExitStack

import concourse.bass as bass
import concourse.tile as tile
from concourse import bacc, mybir
from concourse import bass_utils
from concourse import library_config
from concourse.masks import make_identity

F32 = mybir.dt.float32
F16 = mybir.dt.float16
BF16 = mybir.dt.bfloat16
I16 = mybir.dt.int16
I32 = mybir.dt.int32
AF = mybir.ActivationFunctionType
OP = mybir.AluOpType

CORES = 8
EPS = 1e-30
QCHUNK = 7   # tiles per q dma_gather (<= 896 descriptors, under SWDGE ring)


def _expand_last(ap, n):
    """Append a step-0 (broadcast) innermost dim of size n to an AP."""
    new = [list(p) for p in ap.ap] + [[0, n]]
    return bass.AP(ap.tensor, ap.offset, new)


def _apn(ap, dims, off=0):
    """AP keeping the partition dim of `ap` but custom free-dim pattern."""
    return bass.AP(ap.tensor, ap.offset + off,
                   [list(ap.ap[0])] + [list(d) for d in dims])


def _wrap16(flat):
    """[C, n*128] int -> [C, 128, n*8] int16 in the Q7 gather idx layout:
    idx i at [rep*16 + i%16, i//16], replicated for the 8 Q7 cores."""
    Cd, L = flat.shape
    out = np.zeros((Cd, 128, L // 16), np.int16)
    i = np.arange(L)
    for rep in range(8):
        out[:, rep * 16 + (i % 16), i // 16] = flat
    return out


# ----------------------------------------------------------------- host prep

def _host_prep(x, edge_index):
    N, C = x.shape
    E = edge_index.shape[1]

    NPC = int(math.ceil(N / (CORES * 128))) * 128   # nodes per core (padded)
    BPC = NPC // 128                                # blocks per core
    NPAD = NPC * CORES

    src = edge_index[0].astype(np.int64)
    dst = edge_index[1].astype(np.int64)
    order = np.argsort(dst, kind="stable")
    s_src, s_dst = src[order], dst[order]

    core_of = s_dst // NPC
    blk_of = (s_dst % NPC) // 128
    gblk = core_of * BPC + blk_of

    cnt = np.zeros((CORES, BPC), dtype=np.int64)
    np.add.at(cnt, (core_of, blk_of), 1)
    T_b = np.maximum(1, np.ceil(cnt.max(axis=0) / 128).astype(np.int64))
    tiles_total = int(T_b.sum())

    blk_starts = np.concatenate([[0], np.cumsum(T_b)])[:-1] * 128
    grp_start = np.zeros(CORES * BPC + 1, dtype=np.int64)
    np.add.at(grp_start, gblk + 1, 1)
    grp_start = np.cumsum(grp_start)
    pos_in_grp = np.arange(E) - grp_start[gblk]

    cap = tiles_total * 128
    ekv = np.zeros((CORES, cap), dtype=np.int32)     # global src node id
    eslot = np.full((CORES, cap), -1.0, dtype=np.float32)
    qsl = np.zeros((CORES, cap), dtype=np.int16)     # dst%128 (pad -> 0)

    flat_pos = blk_starts[blk_of] + pos_in_grp
    ekv[core_of, flat_pos] = s_src.astype(np.int32)
    eslot[core_of, flat_pos] = (s_dst % 128).astype(np.float32)
    qsl[core_of, flat_pos] = (s_dst % 128).astype(np.int16)

    # [128, tiles_total] partition-major: entry [p, t] = edge t*128+p
    ekv = ekv.reshape(CORES, tiles_total, 128).transpose(0, 2, 1).copy()
    eslot = np.ascontiguousarray(
        eslot.reshape(CORES, tiles_total, 128).transpose(0, 2, 1)
    ).astype(ml_dtypes.bfloat16)

    meta = dict(N=N, C=C, E=E, NPC=NPC, BPC=BPC, NPAD=NPAD,
                T_b=tuple(int(t) for t in T_b), tiles_total=tiles_total)
    arrays = dict(ekv=ekv, eslot=eslot, qw=_wrap16(qsl))
    return meta, arrays


def _prep_weights(inputs):
    """Fold relation transforms, attention scale and skip gates into weights.

    Stored features are pre-scaled: x_stored = (1-a1)*x, h1_stored = (1-a2)*h1,
    so the skip connection becomes a plain add and the projection weights are
    divided by the input scale.
    """
    C = inputs["W1k"].shape[0]
    H, D = inputs["a1"].shape[0], inputs["a1"].shape[1]
    a_s = {l: float(1.0 / (1.0 + np.exp(-float(np.asarray(inputs[f"skip{l}"])))))
           for l in (1, 2)}
    out = {"asig1": a_s[1], "asig2": a_s[2]}
    names = []
    for l in (1, 2):
        a_rel = np.asarray(inputs[f"a{l}"], np.float64)
        m_rel = np.asarray(inputs[f"m{l}"], np.float64)
        p_rel = np.asarray(inputs[f"p{l}"], np.float64)
        A = np.zeros((C, C)); M = np.zeros((C, C))
        for h in range(H):
            A[h * D:(h + 1) * D, h * D:(h + 1) * D] = a_rel[h]
            M[h * D:(h + 1) * D, h * D:(h + 1) * D] = m_rel[h]
        qscale = np.repeat(p_rel / np.sqrt(D), D)
        in_scale = 1.0 - a_s[l]
        Wq = np.asarray(inputs[f"W{l}q"], np.float64) * qscale / in_scale
        Wk = np.asarray(inputs[f"W{l}k"], np.float64) @ A / in_scale
        Wv = np.asarray(inputs[f"W{l}v"], np.float64) @ M / in_scale
        bq = np.asarray(inputs[f"b{l}q"], np.float64) * qscale
        bk = np.asarray(inputs[f"b{l}k"], np.float64) @ A
        bv = np.asarray(inputs[f"b{l}v"], np.float64) @ M
        out_scale = a_s[1] * (1.0 - a_s[2]) if l == 1 else a_s[2]
        Wo = np.asarray(inputs[f"Wo{l}"], np.float64) * out_scale
        boa = np.asarray(inputs[f"bo{l}"], np.float64) * out_scale
        out[f"Wq{l}"] = Wq.astype(np.float16)
        out[f"Wk{l}"] = Wk.astype(np.float16)
        out[f"Wv{l}"] = Wv.astype(np.float16)
        out[f"Wo{l}"] = Wo.astype(np.float16)
        out[f"bq{l}"] = np.broadcast_to(bq.astype(np.float32), (128, C)).copy()
        out[f"bk{l}"] = np.broadcast_to(bk.astype(np.float32), (128, C)).copy()
        out[f"bv{l}"] = np.broadcast_to(bv.astype(np.float32), (128, C)).copy()
        out[f"boa{l}"] = boa.astype(np.float32).reshape(C, 1).copy()
        names += [f"Wq{l}", f"Wk{l}", f"Wv{l}", f"Wo{l}",
                  f"bq{l}", f"bk{l}", f"bv{l}", f"boa{l}"]
    Wlp = np.asarray(inputs["Wlp"], np.float64)
    out["w12"] = np.stack([Wlp[:C, 0], Wlp[C:, 0]], axis=1).astype(np.float16)
    names.append("w12")
    out["names"] = names
    out["blp"] = float(np.asarray(inputs["blp"]).reshape(-1)[0])
    out["xscale"] = 1.0 - a_s[1]
    return out


# ------------------------------------------------------------------- program

def _build_program(meta, asig1, asig2):
    NPC, BPC, NPAD = meta["NPC"], meta["BPC"], meta["NPAD"]
    T_b, tiles_total = meta["T_b"], meta["tiles_total"]
    Tmax = max(T_b)
    col = np.concatenate([[0], np.cumsum(T_b)]).astype(int)
    C = meta["C"]
    kap = (1.0 - asig2, 1.0)  # skip-add scale on stored input, per layer

    nc = bacc.Bacc("TRN2", target_bir_lowering=False, debug=False,
                   num_devices=CORES, num_swdge_queues=4)

    # --- I/O -------------------------------------------------------------
    xT_in = nc.dram_tensor("xT", [C, NPC], F16, kind="ExternalInput").ap()
    ekv_in = nc.dram_tensor("ekv", [128, tiles_total], I32,
                            kind="ExternalInput").ap()
    eslot_in = nc.dram_tensor("eslot", [128, tiles_total], BF16,
                              kind="ExternalInput").ap()
    qw_in = nc.dram_tensor("qw", [128, tiles_total * 8], I16,
                           kind="ExternalInput").ap()
    wspec = {}
    for l in (1, 2):
        for n in ("Wq", "Wk", "Wv", "Wo"):
            wspec[f"{n}{l}"] = ([128, C], F16)
        for n in ("bq", "bk", "bv"):
            wspec[f"{n}{l}"] = ([128, C], F32)
        wspec[f"boa{l}"] = ([128, 1], F32)
    wspec["w12"] = ([128, 2], F16)
    w_in = {n: nc.dram_tensor(n, s, d, kind="ExternalInput").ap()
            for n, (s, d) in wspec.items()}
    uv_out = nc.dram_tensor("uv_out", [2, NPC], F32, kind="ExternalOutput").ap()

    # --- DRAM scratch ----------------------------------------------------
    q_dram = [nc.dram_tensor(f"q_dram{l}", [NPC, C], F16, kind="Internal").ap()
              for l in (0, 1)]
    kv_shard = [nc.dram_tensor(f"kv_shard{l}", [NPC, 2 * C], F16,
                               kind="Internal").ap() for l in (0, 1)]
    kv_full = [nc.dram_tensor(f"kv_full{l}", [NPAD, 2 * C], F16,
                              kind="Internal").ap() for l in (0, 1)]

    with tile.TileContext(nc) as tc, ExitStack() as ctx:
        cpool = ctx.enter_context(tc.tile_pool(name="const", bufs=1))
        sb = ctx.enter_context(tc.tile_pool(name="sb", bufs=2))
        psum = ctx.enter_context(tc.tile_pool(name="ps", bufs=2, space="PSUM"))

        # --- constants into SBUF ----------------------------------------
        W = {}
        for n, (s, d) in wspec.items():
            W[n] = cpool.tile(s, d, tag=f"w_{n}", name=f"wt_{n}")
            nc.sync.dma_start(W[n][:], w_in[n][:])
        ekv_sb = cpool.tile([128, tiles_total], I32, tag="ekv")
        nc.sync.dma_start(ekv_sb[:], ekv_in[:])
        eslot_sb = cpool.tile([128, tiles_total], BF16, tag="eslot")
        nc.sync.dma_start(eslot_sb[:], eslot_in[:])
        qw_sb = cpool.tile([128, tiles_total * 8], I16, tag="qw")
        nc.sync.dma_start(qw_sb[:], qw_in[:])

        ident = cpool.tile([128, 128], F32, tag="ident")
        make_identity(nc, ident[:])
        iota_i = cpool.tile([128, Tmax * 128], I32, tag="iota_i")
        nc.gpsimd.iota(iota_i[:], pattern=[[0, Tmax], [1, 128]], base=0,
                       channel_multiplier=0)
        iota_bf = cpool.tile([128, Tmax * 128], BF16, tag="iota_bf")
        nc.vector.tensor_copy(iota_bf[:], iota_i[:])
        # dma_gather lives in the mlp Q7 library; iota (standard lib) is done.
        nc.gpsimd.load_library(library_config.mlp)

        xT_all = cpool.tile([128, NPC], F16, tag="xT_all")
        nc.sync.dma_start(xT_all[:], xT_in[:])
        h1T_all = cpool.tile([128, NPC], F16, tag="h1T")
        aggn_all = [cpool.tile([128, NPC], BF16, tag=f"aggn{l}", name=f"aggn{l}")
                    for l in (0, 1)]
        uv_all = cpool.tile([2, NPC], F32, tag="uv")

        qn_state = [0]

        def layer(li, srcT_all):
            l = li + 1
            qd, kvs_d, kvf = q_dram[li], kv_shard[li], kv_full[li]
            # ---- projections for own shard ----
            for b in range(BPC):
                sl = slice(b * 128, (b + 1) * 128)
                lhs = srcT_all[:, sl]
                q_ps = psum.tile([128, C], F32, tag="mm128")
                nc.tensor.matmul(out=q_ps[:], lhsT=lhs, rhs=W[f"Wq{l}"][:],
                                 start=True, stop=True)
                qsb = sb.tile([128, C], F16, tag="qsb")
                nc.vector.tensor_tensor(out=qsb[:], in0=q_ps[:],
                                        in1=W[f"bq{l}"][:], op=OP.add)
                nc.sync.dma_start(qd[sl, :], qsb[:])
                kvs = sb.tile([128, 2 * C], F16, tag="kvs")
                k_ps = psum.tile([128, C], F32, tag="mm128")
                nc.tensor.matmul(out=k_ps[:], lhsT=lhs, rhs=W[f"Wk{l}"][:],
                                 start=True, stop=True)
                nc.vector.tensor_tensor(out=kvs[:, 0:C], in0=k_ps[:],
                                        in1=W[f"bk{l}"][:], op=OP.add)
                v_ps = psum.tile([128, C], F32, tag="mm128")
                nc.tensor.matmul(out=v_ps[:], lhsT=lhs, rhs=W[f"Wv{l}"][:],
                                 start=True, stop=True)
                nc.vector.tensor_tensor(out=kvs[:, C:2 * C], in0=v_ps[:],
                                        in1=W[f"bv{l}"][:], op=OP.add)
                nc.sync.dma_start(kvs_d[sl, :], kvs[:])
            # ---- exchange k/v ----
            nc.gpsimd.collective_compute(
                "AllGather", OP.bypass,
                replica_groups=[list(range(CORES))],
                ins=[kvs_d[:]], outs=[kvf[:]])
            # ---- edge phase ----
            for b in range(BPC):
                T = T_b[b]
                c0 = int(col[b])
                kvg = sb.tile([128, Tmax * 2 * C], F16, tag="kvg")
                for t in range(T):
                    nc.gpsimd.indirect_dma_start(
                        out=kvg[:, t * 256:(t + 1) * 256], out_offset=None,
                        in_=kvf,
                        in_offset=bass.IndirectOffsetOnAxis(
                            ap=ekv_sb[:, c0 + t:c0 + t + 1], axis=0))
                qg = sb.tile([128, Tmax * C], F16, tag="qg")
                for t0 in range(0, T, QCHUNK):
                    G = min(QCHUNK, T - t0)
                    nc.gpsimd.dma_gather(
                        out_ap=_apn(qg[:], [[128, G], [1, 128]],
                                    off=t0 * 128),
                        in_ap=qd[b * 128:(b + 1) * 128, :],
                        idxs_ap=qw_sb[:, (c0 + t0) * 8:(c0 + t0 + G) * 8],
                        num_idxs=G * 128, num_idxs_reg=G * 128,
                        elem_size=128, queue_num=qn_state[0] % 4)
                    qn_state[0] += 1
                S = sb.tile([128, Tmax * 128], BF16, tag="S")
                nc.vector.tensor_tensor(
                    out=_apn(S[:], [[128, T], [1, 128]]),
                    in0=_apn(iota_bf[:], [[128, T], [1, 128]]),
                    in1=_expand_last(eslot_sb[:, c0:c0 + T], 128),
                    op=OP.is_equal)
                prod = sb.tile([128, Tmax * C], F16, tag="prod")
                nc.vector.tensor_tensor(
                    out=_apn(prod[:], [[128, T], [1, 128]]),
                    in0=_apn(kvg[:], [[256, T], [1, 128]]),
                    in1=_apn(qg[:], [[128, T], [1, 128]]), op=OP.mult)
                alpha = sb.tile([128, Tmax * 4], F32, tag="alpha")
                nc.vector.tensor_reduce(
                    out=alpha[:, :T * 4],
                    in_=_apn(prod[:], [[32, T * 4], [1, 32]]),
                    axis=mybir.AxisListType.X, op=OP.add)
                ex = sb.tile([128, Tmax * 4], BF16, tag="ex")
                nc.scalar.activation(ex[:, :T * 4], alpha[:, :T * 4], AF.Exp)
                evex = sb.tile([128, Tmax * 132], BF16, tag="evex")
                nc.scalar.activation(_apn(evex[:], [[132, T], [1, 4]]),
                                     alpha[:, :T * 4], AF.Exp)
                nc.vector.tensor_tensor(
                    out=_apn(evex[:], [[132, T], [32, 4], [1, 32]], off=4),
                    in0=_apn(kvg[:], [[256, T], [32, 4], [1, 32]], off=128),
                    in1=_apn(ex[:], [[4, T], [1, 4], [0, 32]]), op=OP.mult)
                agg_ps = psum.tile([128, 132], F32, tag="mm132")
                for t in range(T):
                    nc.tensor.matmul(out=agg_ps[:],
                                     lhsT=S[:, t * 128:(t + 1) * 128],
                                     rhs=evex[:, t * 132:(t + 1) * 132],
                                     start=(t == 0), stop=(t == T - 1))
                den = sb.tile([128, 4], F32, tag="den")
                nc.vector.tensor_scalar_add(den[:], agg_ps[:, 0:4], EPS)
                rd = sb.tile([128, 4], F32, tag="rd")
                nc.vector.reciprocal(rd[:], den[:])
                nc.vector.tensor_tensor(
                    out=_apn(aggn_all[li][:], [[32, 4], [1, 32]], off=b * 128),
                    in0=_apn(agg_ps[:], [[32, 4], [1, 32]], off=4),
                    in1=_expand_last(rd[:], 32), op=OP.mult)
            # ---- pass 2: gelu, output projection, skip (+ decode) ----
            for b in range(BPC):
                sl = slice(b * 128, (b + 1) * 128)
                g = sb.tile([128, C], F32, tag="g")
                nc.scalar.activation(g[:], aggn_all[li][:, sl], AF.Gelu)
                gT_ps = psum.tile([128, C], F32, tag="tr")
                nc.tensor.transpose(out=gT_ps[:], in_=g[:], identity=ident[:])
                gT = sb.tile([128, C], BF16, tag="gTs")
                nc.vector.tensor_copy(gT[:], gT_ps[:])
                hm_ps = psum.tile([128, C], F32, tag="mm128")
                nc.tensor.matmul(out=hm_ps[:], lhsT=W[f"Wo{l}"][:], rhs=gT[:],
                                 start=True, stop=True)
                if l == 1:
                    zt_ap = h1T_all[:, sl]
                else:
                    zt = sb.tile([128, C], F16, tag="zt")
                    zt_ap = zt[:]
                nc.vector.scalar_tensor_tensor(
                    out=zt_ap, in0=srcT_all[:, sl], scalar=kap[li],
                    in1=hm_ps[:], op0=OP.mult, op1=OP.add)
                nc.vector.tensor_tensor(
                    out=zt_ap, in0=zt_ap,
                    in1=_apn(W[f"boa{l}"][:], [[0, 128]]), op=OP.add)
                if l == 2:
                    uv_ps = psum.tile([128, C], F32, tag="mm128")
                    nc.tensor.matmul(out=uv_ps[0:2, :], lhsT=W["w12"][:],
                                     rhs=zt_ap, start=True, stop=True)
                    nc.vector.tensor_copy(uv_all[:, sl], uv_ps[0:2, :])

        layer(0, xT_all)
        layer(1, h1T_all)
        nc.sync.dma_start(uv_out, uv_all[:])

    nc.compile()
    return nc


_CACHE = {}


def _get_program(meta, asig1, asig2, blp):
    key = (meta["N"], meta["E"], meta["T_b"], asig1, asig2)
    if key not in _CACHE:
        _CACHE[key] = _build_program(meta, asig1, asig2)
    return _CACHE[key]


def make_in_maps(inputs):
    inputs = {k: np.asarray(v) for k, v in inputs.items()}
    meta, arrays = _host_prep(np.asarray(inputs["x"], np.float32),
                              inputs["edge_index"])
    w = _prep_weights(inputs)
    N, C, NPC = meta["N"], meta["C"], meta["NPC"]
    xpad = np.zeros((meta["NPAD"], C), dtype=np.float64)
    xpad[:N] = np.asarray(inputs["x"], np.float64)
    xT_full = np.ascontiguousarray((xpad * w["xscale"]).T).astype(np.float16)
    in_maps = []
    for c in range(CORES):
        m = dict(xT=np.ascontiguousarray(xT_full[:, c * NPC:(c + 1) * NPC]),
                 ekv=arrays["ekv"][c], eslot=arrays["eslot"][c],
                 qw=arrays["qw"][c])
        for n in w["names"]:
            m[n] = w[n]
        in_maps.append(m)
    return meta, w, in_maps


def assemble(meta, results, inputs, blp):
    u = np.concatenate([results[c]["uv_out"] for c in range(CORES)], axis=1)
    u1, u2 = u[0], u[1]
    pe, ne = inputs["pos_edge_index"], inputs["neg_edge_index"]
    pos = u1[pe[0]] + u2[pe[1]] + np.float32(blp)
    neg = u1[ne[0]] + u2[ne[1]] + np.float32(blp)
    return pos.astype(np.float32), neg.astype(np.float32)


def kernel(**inputs):
    meta, w, in_maps = make_in_maps(inputs)
    nc = _get_program(meta, w["asig1"], w["asig2"], w["blp"])
    res = bass_utils.run_bass_kernel_spmd(nc, in_maps,
                                          core_ids=list(range(CORES)))
    return assemble(meta, res.results, inputs, w["blp"])


# revision 9
# speedup vs baseline: 2.5535x; 1.3887x over previous
"""HGT link predictor on 8 Trainium2 NeuronCores (Bass/Tile SPMD kernel).

Strategy (hardcoded for nn_HGTLinkPredictor, N=50000 E=800000 P=100000 C=128 H=4 D=32):
 - Shard dst nodes (and their incoming edges) across 8 cores in contiguous
   128-node blocks. Edges sorted by dst on host.
 - Per layer: each core computes q/k/v projections for its node shard from a
   host-transposed feature tile (relation transforms, attention scale and the
   sigmoid-skip coefficients are all folded into the weights on host), writes
   packed [k|v] fp16 rows, AllGathers them so every core can fetch k/v of any
   src node with a single 512B-descriptor indirect DMA per edge (one packed
   row instead of separate k/v/q fetches).
 - q[dst] rows are fetched with batched Q7 dma_gather instructions whose
   indices are block-relative (dst%128) — the gather ucode routes indices
   through an fp16 conversion, so only small indices are exact.
 - Attention logits via fp16 elementwise mult + segmented reduce; exp on the
   scalar engine into a fused [ex | v*ex] bf16 tile so a single 132-column
   matmul per 128-edge tile accumulates both the softmax denominator and the
   weighted value sum in PSUM.
 - Gelu + output projection run as a deferred second pass (one activation
   table load), producing transposed features directly so layer-2 projections
   need no PE transposes. Link decode is one tiny matmul per block; final
   per-edge logits are assembled host-side from per-node partial sums.
"""

import math
import numpy as np
import ml_dtypes
from contextlib import ExitStack

import concourse.bass as bass
import concourse.tile as tile
from concourse import bacc, mybir
from concourse import bass_utils
from concourse import library_config
from concourse.masks import make_identity

F32 = mybir.dt.float32
F16 = mybir.dt.float16
BF16 = mybir.dt.bfloat16
I16 = mybir.dt.int16
I32 = mybir.dt.int32
AF = mybir.ActivationFunctionType
OP = mybir.AluOpType

CORES = 8
EPS = 1e-30
QCHUNK = 7   # tiles per q dma_gather (<= 896 descriptors, under SWDGE ring)


def _expand_last(ap, n):
    """Append a step-0 (broadcast) innermost dim of size n to an AP."""
    new = [list(p) for p in ap.ap] + [[0, n]]
    return bass.AP(ap.tensor, ap.offset, new)


def _apn(ap, dims, off=0):
    """AP keeping the partition dim of `ap` but custom free-dim pattern."""
    return bass.AP(ap.tensor, ap.offset + off,
                   [list(ap.ap[0])] + [list(d) for d in dims])


def _wrap16(flat):
    """[C, n*128] int -> [C, 128, n*8] int16 in the Q7 gather idx layout:
    idx i at [rep*16 + i%16, i//16], replicated for the 8 Q7 cores."""
    Cd, L = flat.shape
    out = np.zeros((Cd, 128, L // 16), np.int16)
    i = np.arange(L)
    for rep in range(8):
        out[:, rep * 16 + (i % 16), i // 16] = flat
    return out


# ----------------------------------------------------------------- host prep

def _host_prep(x, edge_index):
    N, C = x.shape
    E = edge_index.shape[1]

    NPC = int(math.ceil(N / (CORES * 128))) * 128   # nodes per core (padded)
    BPC = NPC // 128                                # blocks per core
    NPAD = NPC * CORES

    src = edge_index[0].astype(np.int64)
    dst = edge_index[1].astype(np.int64)
    order = np.argsort(dst, kind="stable")
    s_src, s_dst = src[order], dst[order]

    core_of = s_dst // NPC
    blk_of = (s_dst % NPC) // 128
    gblk = core_of * BPC + blk_of

    cnt = np.zeros((CORES, BPC), dtype=np.int64)
    np.add.at(cnt, (core_of, blk_of), 1)
    T_b = np.maximum(1, np.ceil(cnt.max(axis=0) / 128).astype(np.int64))
    tiles_total = int(T_b.sum())

    blk_starts = np.concatenate([[0], np.cumsum(T_b)])[:-1] * 128
    grp_start = np.zeros(CORES * BPC + 1, dtype=np.int64)
    np.add.at(grp_start, gblk + 1, 1)
    grp_start = np.cumsum(grp_start)
    pos_in_grp = np.arange(E) - grp_start[gblk]

    cap = tiles_total * 128
    ekv = np.zeros((CORES, cap), dtype=np.int32)     # global src node id
    eslot = np.full((CORES, cap), -1.0, dtype=np.float32)

    flat_pos = blk_starts[blk_of] + pos_in_grp
    ekv[core_of, flat_pos] = s_src.astype(np.int32)
    eslot[core_of, flat_pos] = (s_dst % 128).astype(np.float32)

    # transposed one-hot: S2[j, t*128+e] = (eslot[t*128+e] == j), bf16
    s2 = (eslot.reshape(CORES, 1, cap)
          == np.arange(128, dtype=np.float32)[None, :, None])
    s2 = s2.astype(ml_dtypes.bfloat16)
    # [128, tiles_total] partition-major: entry [p, t] = edge t*128+p
    ekv = ekv.reshape(CORES, tiles_total, 128).transpose(0, 2, 1).copy()
    eslot = np.ascontiguousarray(
        eslot.reshape(CORES, tiles_total, 128).transpose(0, 2, 1)
    ).astype(ml_dtypes.bfloat16)

    meta = dict(N=N, C=C, E=E, NPC=NPC, BPC=BPC, NPAD=NPAD,
                T_b=tuple(int(t) for t in T_b), tiles_total=tiles_total)
    arrays = dict(ekv=ekv, eslot=eslot, s2=s2)
    return meta, arrays


def _prep_weights(inputs):
    """Fold relation transforms, attention scale and skip gates into weights.

    Stored features are pre-scaled: x_stored = (1-a1)*x, h1_stored = (1-a2)*h1,
    so the skip connection becomes a plain add and the projection weights are
    divided by the input scale.
    """
    C = inputs["W1k"].shape[0]
    H, D = inputs["a1"].shape[0], inputs["a1"].shape[1]
    a_s = {l: float(1.0 / (1.0 + np.exp(-float(np.asarray(inputs[f"skip{l}"])))))
           for l in (1, 2)}
    out = {"asig1": a_s[1], "asig2": a_s[2]}
    names = []
    for l in (1, 2):
        a_rel = np.asarray(inputs[f"a{l}"], np.float64)
        m_rel = np.asarray(inputs[f"m{l}"], np.float64)
        p_rel = np.asarray(inputs[f"p{l}"], np.float64)
        A = np.zeros((C, C)); M = np.zeros((C, C))
        for h in range(H):
            A[h * D:(h + 1) * D, h * D:(h + 1) * D] = a_rel[h]
            M[h * D:(h + 1) * D, h * D:(h + 1) * D] = m_rel[h]
        qscale = np.repeat(p_rel / np.sqrt(D), D)
        in_scale = 1.0 - a_s[l]
        Wq = np.asarray(inputs[f"W{l}q"], np.float64) * qscale / in_scale
        Wk = np.asarray(inputs[f"W{l}k"], np.float64) @ A / in_scale
        Wv = np.asarray(inputs[f"W{l}v"], np.float64) @ M / in_scale
        bq = np.asarray(inputs[f"b{l}q"], np.float64) * qscale
        bk = np.asarray(inputs[f"b{l}k"], np.float64) @ A
        bv = np.asarray(inputs[f"b{l}v"], np.float64) @ M
        out_scale = a_s[1] * (1.0 - a_s[2]) if l == 1 else a_s[2]
        Wo = np.asarray(inputs[f"Wo{l}"], np.float64) * out_scale
        boa = np.asarray(inputs[f"bo{l}"], np.float64) * out_scale
        out[f"Wq{l}"] = Wq.astype(np.float16)
        out[f"Wk{l}"] = Wk.astype(np.float16)
        out[f"Wv{l}"] = Wv.astype(np.float16)
        out[f"Wo{l}"] = Wo.astype(np.float16)
        out[f"bq{l}"] = np.broadcast_to(bq.astype(np.float32), (128, C)).copy()
        out[f"bk{l}"] = np.broadcast_to(bk.astype(np.float32), (128, C)).copy()
        out[f"bv{l}"] = np.broadcast_to(bv.astype(np.float32), (128, C)).copy()
        out[f"boa{l}"] = boa.astype(np.float32).reshape(C, 1).copy()
        names += [f"Wq{l}", f"Wk{l}", f"Wv{l}", f"Wo{l}",
                  f"bq{l}", f"bk{l}", f"bv{l}", f"boa{l}"]
    Wlp = np.asarray(inputs["Wlp"], np.float64)
    out["w12"] = np.stack([Wlp[:C, 0], Wlp[C:, 0]], axis=1).astype(np.float16)
    names.append("w12")
    out["names"] = names
    out["blp"] = float(np.asarray(inputs["blp"]).reshape(-1)[0])
    out["xscale"] = 1.0 - a_s[1]
    return out


# ------------------------------------------------------------------- program

def _build_program(meta, asig1, asig2):
    NPC, BPC, NPAD = meta["NPC"], meta["BPC"], meta["NPAD"]
    T_b, tiles_total = meta["T_b"], meta["tiles_total"]
    Tmax = max(T_b)
    col = np.concatenate([[0], np.cumsum(T_b)]).astype(int)
    C = meta["C"]
    kap = (1.0 - asig2, 1.0)  # skip-add scale on stored input, per layer

    nc = bacc.Bacc("TRN2", target_bir_lowering=False, debug=False,
                   num_devices=CORES)

    # --- I/O -------------------------------------------------------------
    xT_in = nc.dram_tensor("xT", [C, NPC], F16, kind="ExternalInput").ap()
    ekv_in = nc.dram_tensor("ekv", [128, tiles_total], I32,
                            kind="ExternalInput").ap()
    eslot_in = nc.dram_tensor("eslot", [128, tiles_total], BF16,
                              kind="ExternalInput").ap()
    s2_in = nc.dram_tensor("s2", [128, tiles_total * 128], BF16,
                           kind="ExternalInput").ap()
    wspec = {}
    for l in (1, 2):
        for n in ("Wq", "Wk", "Wv", "Wo"):
            wspec[f"{n}{l}"] = ([128, C], F16)
        for n in ("bq", "bk", "bv"):
            wspec[f"{n}{l}"] = ([128, C], F32)
        wspec[f"boa{l}"] = ([128, 1], F32)
    wspec["w12"] = ([128, 2], F16)
    w_in = {n: nc.dram_tensor(n, s, d, kind="ExternalInput").ap()
            for n, (s, d) in wspec.items()}
    uv_out = nc.dram_tensor("uv_out", [2, NPC], F32, kind="ExternalOutput").ap()

    # --- DRAM scratch ----------------------------------------------------
    kv_shard = [nc.dram_tensor(f"kv_shard{l}", [NPC, 2 * C], F16,
                               kind="Internal").ap() for l in (0, 1)]
    kv_full = [nc.dram_tensor(f"kv_full{l}", [NPAD, 2 * C], F16,
                              kind="Internal").ap() for l in (0, 1)]

    with tile.TileContext(nc) as tc, ExitStack() as ctx:
        cpool = ctx.enter_context(tc.tile_pool(name="const", bufs=1))
        sb = ctx.enter_context(tc.tile_pool(name="sb", bufs=2))
        psum = ctx.enter_context(tc.tile_pool(name="ps", bufs=2, space="PSUM"))

        # --- constants into SBUF ----------------------------------------
        W = {}
        for n, (s, d) in wspec.items():
            W[n] = cpool.tile(s, d, tag=f"w_{n}", name=f"wt_{n}")
            nc.sync.dma_start(W[n][:], w_in[n][:])
        ekv_sb = cpool.tile([128, tiles_total], I32, tag="ekv")
        nc.sync.dma_start(ekv_sb[:], ekv_in[:])
        eslot_sb = cpool.tile([128, tiles_total], BF16, tag="eslot")
        nc.sync.dma_start(eslot_sb[:], eslot_in[:])

        ident = cpool.tile([128, 128], F32, tag="ident")
        make_identity(nc, ident[:])
        iota_i = cpool.tile([128, Tmax * 128], I32, tag="iota_i")
        nc.gpsimd.iota(iota_i[:], pattern=[[0, Tmax], [1, 128]], base=0,
                       channel_multiplier=0)
        iota_bf = cpool.tile([128, Tmax * 128], BF16, tag="iota_bf")
        nc.vector.tensor_copy(iota_bf[:], iota_i[:])

        xT_all = cpool.tile([128, NPC], F16, tag="xT_all")
        nc.sync.dma_start(xT_all[:], xT_in[:])
        h1T_all = cpool.tile([128, NPC], F16, tag="h1T")
        aggn_all = [cpool.tile([128, NPC], BF16, tag=f"aggn{l}", name=f"aggn{l}")
                    for l in (0, 1)]
        uv_all = cpool.tile([2, NPC], F32, tag="uv")
        q_all = [cpool.tile([128, NPC], F16, tag=f"q_all{l}", name=f"q_all{l}")
                 for l in (0, 1)]

        def layer(li, srcT_all):
            l = li + 1
            kvs_d, kvf = kv_shard[li], kv_full[li]
            qa = q_all[li]
            # ---- projections for own shard ----
            for b in range(BPC):
                sl = slice(b * 128, (b + 1) * 128)
                lhs = srcT_all[:, sl]
                q_ps = psum.tile([128, C], F32, tag="mm128")
                nc.tensor.matmul(out=q_ps[:], lhsT=lhs, rhs=W[f"Wq{l}"][:],
                                 start=True, stop=True)
                nc.vector.tensor_tensor(out=qa[:, sl], in0=q_ps[:],
                                        in1=W[f"bq{l}"][:], op=OP.add)
                kvs = sb.tile([128, 2 * C], F16, tag="kvs")
                k_ps = psum.tile([128, C], F32, tag="mm128")
                nc.tensor.matmul(out=k_ps[:], lhsT=lhs, rhs=W[f"Wk{l}"][:],
                                 start=True, stop=True)
                nc.vector.tensor_tensor(out=kvs[:, 0:C], in0=k_ps[:],
                                        in1=W[f"bk{l}"][:], op=OP.add)
                v_ps = psum.tile([128, C], F32, tag="mm128")
                nc.tensor.matmul(out=v_ps[:], lhsT=lhs, rhs=W[f"Wv{l}"][:],
                                 start=True, stop=True)
                nc.vector.tensor_tensor(out=kvs[:, C:2 * C], in0=v_ps[:],
                                        in1=W[f"bv{l}"][:], op=OP.add)
                nc.sync.dma_start(kvs_d[sl, :], kvs[:])
            # ---- exchange k/v ----
            nc.gpsimd.collective_compute(
                "AllGather", OP.bypass,
                replica_groups=[list(range(CORES))],
                ins=[kvs_d[:]], outs=[kvf[:]])
            # ---- edge phase ----
            for b in range(BPC):
                T = T_b[b]
                c0 = int(col[b])
                kvg = sb.tile([128, Tmax * 2 * C], F16, tag="kvg")
                for t in range(T):
                    nc.gpsimd.indirect_dma_start(
                        out=kvg[:, t * 256:(t + 1) * 256], out_offset=None,
                        in_=kvf,
                        in_offset=bass.IndirectOffsetOnAxis(
                            ap=ekv_sb[:, c0 + t:c0 + t + 1], axis=0))
                S2 = sb.tile([128, Tmax * 128], BF16, tag="S2")
                nc.sync.dma_start(
                    S2[:, :T * 128],
                    s2_in[:, c0 * 128:(c0 + T) * 128])
                qg = sb.tile([128, Tmax * C], F16, tag="qg")
                for t in range(T):
                    qg_ps = psum.tile([128, C], F32, tag="qg")
                    nc.tensor.matmul(out=qg_ps[:],
                                     lhsT=S2[:, t * 128:(t + 1) * 128],
                                     rhs=qa[:, b * 128:(b + 1) * 128],
                                     start=True, stop=True)
                    if t % 2 == 0:
                        nc.scalar.activation(qg[:, t * 128:(t + 1) * 128],
                                             qg_ps[:], AF.Copy)
                    else:
                        nc.vector.tensor_copy(qg[:, t * 128:(t + 1) * 128],
                                              qg_ps[:])
                S = sb.tile([128, Tmax * 128], BF16, tag="S")
                nc.vector.tensor_tensor(
                    out=_apn(S[:], [[128, T], [1, 128]]),
                    in0=_apn(iota_bf[:], [[128, T], [1, 128]]),
                    in1=_expand_last(eslot_sb[:, c0:c0 + T], 128),
                    op=OP.is_equal)
                prod = sb.tile([128, Tmax * C], F16, tag="prod")
                nc.vector.tensor_tensor(
                    out=_apn(prod[:], [[128, T], [1, 128]]),
                    in0=_apn(kvg[:], [[256, T], [1, 128]]),
                    in1=_apn(qg[:], [[128, T], [1, 128]]), op=OP.mult)
                alpha = sb.tile([128, Tmax * 4], F32, tag="alpha")
                nc.vector.tensor_reduce(
                    out=alpha[:, :T * 4],
                    in_=_apn(prod[:], [[32, T * 4], [1, 32]]),
                    axis=mybir.AxisListType.X, op=OP.add)
                ex = sb.tile([128, Tmax * 4], BF16, tag="ex")
                nc.scalar.activation(ex[:, :T * 4], alpha[:, :T * 4], AF.Exp)
                evex = sb.tile([128, Tmax * 132], BF16, tag="evex")
                nc.scalar.activation(_apn(evex[:], [[132, T], [1, 4]]),
                                     alpha[:, :T * 4], AF.Exp)
                nc.vector.tensor_tensor(
                    out=_apn(evex[:], [[132, T], [32, 4], [1, 32]], off=4),
                    in0=_apn(kvg[:], [[256, T], [32, 4], [1, 32]], off=128),
                    in1=_apn(ex[:], [[4, T], [1, 4], [0, 32]]), op=OP.mult)
                agg_ps = psum.tile([128, 132], F32, tag="mm132")
                for t in range(T):
                    nc.tensor.matmul(out=agg_ps[:],
                                     lhsT=S[:, t * 128:(t + 1) * 128],
                                     rhs=evex[:, t * 132:(t + 1) * 132],
                                     start=(t == 0), stop=(t == T - 1))
                den = sb.tile([128, 4], F32, tag="den")
                nc.vector.tensor_scalar_add(den[:], agg_ps[:, 0:4], EPS)
                rd = sb.tile([128, 4], F32, tag="rd")
                nc.vector.reciprocal(rd[:], den[:])
                nc.vector.tensor_tensor(
                    out=_apn(aggn_all[li][:], [[32, 4], [1, 32]], off=b * 128),
                    in0=_apn(agg_ps[:], [[32, 4], [1, 32]], off=4),
                    in1=_expand_last(rd[:], 32), op=OP.mult)
            # ---- pass 2: gelu, output projection, skip (+ decode) ----
            for b in range(BPC):
                sl = slice(b * 128, (b + 1) * 128)
                g = sb.tile([128, C], F32, tag="g")
                nc.scalar.activation(g[:], aggn_all[li][:, sl], AF.Gelu)
                gT_ps = psum.tile([128, C], F32, tag="tr")
                nc.tensor.transpose(out=gT_ps[:], in_=g[:], identity=ident[:])
                gT = sb.tile([128, C], BF16, tag="gTs")
                nc.vector.tensor_copy(gT[:], gT_ps[:])
                hm_ps = psum.tile([128, C], F32, tag="mm128")
                nc.tensor.matmul(out=hm_ps[:], lhsT=W[f"Wo{l}"][:], rhs=gT[:],
                                 start=True, stop=True)
                if l == 1:
                    zt_ap = h1T_all[:, sl]
                else:
                    zt = sb.tile([128, C], F16, tag="zt")
                    zt_ap = zt[:]
                nc.vector.scalar_tensor_tensor(
                    out=zt_ap, in0=srcT_all[:, sl], scalar=kap[li],
                    in1=hm_ps[:], op0=OP.mult, op1=OP.add)
                nc.vector.tensor_tensor(
                    out=zt_ap, in0=zt_ap,
                    in1=_apn(W[f"boa{l}"][:], [[0, 128]]), op=OP.add)
                if l == 2:
                    uv_ps = psum.tile([128, C], F32, tag="mm128")
                    nc.tensor.matmul(out=uv_ps[0:2, :], lhsT=W["w12"][:],
                                     rhs=zt_ap, start=True, stop=True)
                    nc.vector.tensor_copy(uv_all[:, sl], uv_ps[0:2, :])

        layer(0, xT_all)
        layer(1, h1T_all)
        nc.sync.dma_start(uv_out, uv_all[:])

    nc.compile()
    return nc


_CACHE = {}


def _get_program(meta, asig1, asig2, blp):
    key = (meta["N"], meta["E"], meta["T_b"], asig1, asig2)
    if key not in _CACHE:
        _CACHE[key] = _build_program(meta, asig1, asig2)
    return _CACHE[key]


def make_in_maps(inputs):
    inputs = {k: np.asarray(v) for k, v in inputs.items()}
    meta, arrays = _host_prep(np.asarray(inputs["x"], np.float32),
                              inputs["edge_index"])
    w = _prep_weights(inputs)
    N, C, NPC = meta["N"], meta["C"], meta["NPC"]
    xpad = np.zeros((meta["NPAD"], C), dtype=np.float64)
    xpad[:N] = np.asarray(inputs["x"], np.float64)
    xT_full = np.ascontiguousarray((xpad * w["xscale"]).T).astype(np.float16)
    in_maps = []
    for c in range(CORES):
        m = dict(xT=np.ascontiguousarray(xT_full[:, c * NPC:(c + 1) * NPC]),
                 ekv=arrays["ekv"][c], eslot=arrays["eslot"][c],
                 s2=arrays["s2"][c])
        for n in w["names"]:
            m[n] = w[n]
        in_maps.append(m)
    return meta, w, in_maps


def assemble(meta, results, inputs, blp):
    u = np.concatenate([results[c]["uv_out"] for c in range(CORES)], axis=1)
    u1, u2 = u[0], u[1]
    pe, ne = inputs["pos_edge_index"], inputs["neg_edge_index"]
    pos = u1[pe[0]] + u2[pe[1]] + np.float32(blp)
    neg = u1[ne[0]] + u2[ne[1]] + np.float32(blp)
    return pos.astype(np.float32), neg.astype(np.float32)


def kernel(**inputs):
    meta, w, in_maps = make_in_maps(inputs)
    nc = _get_program(meta, w["asig1"], w["asig2"], w["blp"])
    res = bass_utils.run_bass_kernel_spmd(nc, in_maps,
                                          core_ids=list(range(CORES)))
    return assemble(meta, res.results, inputs, w["blp"])


# revision 10
# speedup vs baseline: 2.5962x; 1.0167x over previous
"""HGT link predictor on 8 Trainium2 NeuronCores (Bass/Tile SPMD kernel).

Strategy (hardcoded for nn_HGTLinkPredictor, N=50000 E=800000 P=100000 C=128 H=4 D=32):
 - Shard dst nodes (and their incoming edges) across 8 cores in contiguous
   128-node blocks. Edges sorted by dst on host.
 - Per layer: each core computes q/k/v projections for its node shard from a
   host-transposed feature tile (relation transforms, attention scale and the
   sigmoid-skip coefficients are all folded into the weights on host), writes
   packed [k|v] fp16 rows, AllGathers them so every core can fetch k/v of any
   src node with a single 512B-descriptor indirect DMA per edge (one packed
   row instead of separate k/v/q fetches).
 - q[dst] rows are fetched with batched Q7 dma_gather instructions whose
   indices are block-relative (dst%128) — the gather ucode routes indices
   through an fp16 conversion, so only small indices are exact.
 - Attention logits via fp16 elementwise mult + segmented reduce; exp on the
   scalar engine into a fused [ex | v*ex] bf16 tile so a single 132-column
   matmul per 128-edge tile accumulates both the softmax denominator and the
   weighted value sum in PSUM.
 - Gelu + output projection run as a deferred second pass (one activation
   table load), producing transposed features directly so layer-2 projections
   need no PE transposes. Link decode is one tiny matmul per block; final
   per-edge logits are assembled host-side from per-node partial sums.
"""

import math
import numpy as np
import ml_dtypes
from contextlib import ExitStack

import concourse.bass as bass
import concourse.tile as tile
from concourse import bacc, mybir
from concourse import bass_utils
from concourse import library_config
from concourse.masks import make_identity

F32 = mybir.dt.float32
F16 = mybir.dt.float16
BF16 = mybir.dt.bfloat16
I16 = mybir.dt.int16
I32 = mybir.dt.int32
AF = mybir.ActivationFunctionType
OP = mybir.AluOpType

CORES = 8
EPS = 1e-30
QCHUNK = 7   # tiles per q dma_gather (<= 896 descriptors, under SWDGE ring)


def _expand_last(ap, n):
    """Append a step-0 (broadcast) innermost dim of size n to an AP."""
    new = [list(p) for p in ap.ap] + [[0, n]]
    return bass.AP(ap.tensor, ap.offset, new)


def _apn(ap, dims, off=0):
    """AP keeping the partition dim of `ap` but custom free-dim pattern."""
    return bass.AP(ap.tensor, ap.offset + off,
                   [list(ap.ap[0])] + [list(d) for d in dims])


def _wrap16(flat):
    """[C, n*128] int -> [C, 128, n*8] int16 in the Q7 gather idx layout:
    idx i at [rep*16 + i%16, i//16], replicated for the 8 Q7 cores."""
    Cd, L = flat.shape
    out = np.zeros((Cd, 128, L // 16), np.int16)
    i = np.arange(L)
    for rep in range(8):
        out[:, rep * 16 + (i % 16), i // 16] = flat
    return out


# ----------------------------------------------------------------- host prep

def _host_prep(x, edge_index):
    N, C = x.shape
    E = edge_index.shape[1]

    NPC = int(math.ceil(N / (CORES * 128))) * 128   # nodes per core (padded)
    BPC = NPC // 128                                # blocks per core
    NPAD = NPC * CORES

    src = edge_index[0].astype(np.int64)
    dst = edge_index[1].astype(np.int64)
    order = np.argsort(dst, kind="stable")
    s_src, s_dst = src[order], dst[order]

    core_of = s_dst // NPC
    blk_of = (s_dst % NPC) // 128
    gblk = core_of * BPC + blk_of

    cnt = np.zeros((CORES, BPC), dtype=np.int64)
    np.add.at(cnt, (core_of, blk_of), 1)
    T_b = np.maximum(1, np.ceil(cnt.max(axis=0) / 128).astype(np.int64))
    tiles_total = int(T_b.sum())

    blk_starts = np.concatenate([[0], np.cumsum(T_b)])[:-1] * 128
    grp_start = np.zeros(CORES * BPC + 1, dtype=np.int64)
    np.add.at(grp_start, gblk + 1, 1)
    grp_start = np.cumsum(grp_start)
    pos_in_grp = np.arange(E) - grp_start[gblk]

    cap = tiles_total * 128
    ekv = np.zeros((CORES, cap), dtype=np.int32)     # global src node id
    eslot = np.full((CORES, cap), -1.0, dtype=np.float32)

    flat_pos = blk_starts[blk_of] + pos_in_grp
    ekv[core_of, flat_pos] = s_src.astype(np.int32)
    eslot[core_of, flat_pos] = (s_dst % 128).astype(np.float32)

    # transposed one-hot: S2[j, t*128+e] = (eslot[t*128+e] == j), bf16
    s2 = (eslot.reshape(CORES, 1, cap)
          == np.arange(128, dtype=np.float32)[None, :, None])
    s2 = s2.astype(ml_dtypes.bfloat16)
    # [128, tiles_total] partition-major: entry [p, t] = edge t*128+p
    ekv = ekv.reshape(CORES, tiles_total, 128).transpose(0, 2, 1).copy()
    eslot = np.ascontiguousarray(
        eslot.reshape(CORES, tiles_total, 128).transpose(0, 2, 1)
    ).astype(ml_dtypes.bfloat16)

    meta = dict(N=N, C=C, E=E, NPC=NPC, BPC=BPC, NPAD=NPAD,
                T_b=tuple(int(t) for t in T_b), tiles_total=tiles_total)
    arrays = dict(ekv=ekv, eslot=eslot, s2=s2)
    return meta, arrays


def _prep_weights(inputs):
    """Fold relation transforms, attention scale and skip gates into weights.

    Stored features are pre-scaled: x_stored = (1-a1)*x, h1_stored = (1-a2)*h1,
    so the skip connection becomes a plain add and the projection weights are
    divided by the input scale.
    """
    C = inputs["W1k"].shape[0]
    H, D = inputs["a1"].shape[0], inputs["a1"].shape[1]
    a_s = {l: float(1.0 / (1.0 + np.exp(-float(np.asarray(inputs[f"skip{l}"])))))
           for l in (1, 2)}
    out = {"asig1": a_s[1], "asig2": a_s[2]}
    names = []
    for l in (1, 2):
        a_rel = np.asarray(inputs[f"a{l}"], np.float64)
        m_rel = np.asarray(inputs[f"m{l}"], np.float64)
        p_rel = np.asarray(inputs[f"p{l}"], np.float64)
        A = np.zeros((C, C)); M = np.zeros((C, C))
        for h in range(H):
            A[h * D:(h + 1) * D, h * D:(h + 1) * D] = a_rel[h]
            M[h * D:(h + 1) * D, h * D:(h + 1) * D] = m_rel[h]
        qscale = np.repeat(p_rel / np.sqrt(D), D)
        in_scale = 1.0 - a_s[l]
        Wq = np.asarray(inputs[f"W{l}q"], np.float64) * qscale / in_scale
        Wk = np.asarray(inputs[f"W{l}k"], np.float64) @ A / in_scale
        Wv = np.asarray(inputs[f"W{l}v"], np.float64) @ M / in_scale
        bq = np.asarray(inputs[f"b{l}q"], np.float64) * qscale
        bk = np.asarray(inputs[f"b{l}k"], np.float64) @ A
        bv = np.asarray(inputs[f"b{l}v"], np.float64) @ M
        out_scale = a_s[1] * (1.0 - a_s[2]) if l == 1 else a_s[2]
        Wo = np.asarray(inputs[f"Wo{l}"], np.float64) * out_scale
        boa = np.asarray(inputs[f"bo{l}"], np.float64) * out_scale
        out[f"Wq{l}"] = Wq.astype(np.float16)
        out[f"Wk{l}"] = Wk.astype(np.float16)
        out[f"Wv{l}"] = Wv.astype(np.float16)
        out[f"Wo{l}"] = Wo.astype(np.float16)
        out[f"bq{l}"] = np.broadcast_to(bq.astype(np.float32), (128, C)).copy()
        out[f"bk{l}"] = np.broadcast_to(bk.astype(np.float32), (128, C)).copy()
        out[f"bv{l}"] = np.broadcast_to(bv.astype(np.float32), (128, C)).copy()
        out[f"boa{l}"] = boa.astype(np.float32).reshape(C, 1).copy()
        names += [f"Wq{l}", f"Wk{l}", f"Wv{l}", f"Wo{l}",
                  f"bq{l}", f"bk{l}", f"bv{l}", f"boa{l}"]
    Wlp = np.asarray(inputs["Wlp"], np.float64)
    out["w12"] = np.stack([Wlp[:C, 0], Wlp[C:, 0]], axis=1).astype(np.float16)
    names.append("w12")
    out["names"] = names
    out["blp"] = float(np.asarray(inputs["blp"]).reshape(-1)[0])
    out["xscale"] = 1.0 - a_s[1]
    return out


# ------------------------------------------------------------------- program

def _build_program(meta, asig1, asig2):
    NPC, BPC, NPAD = meta["NPC"], meta["BPC"], meta["NPAD"]
    T_b, tiles_total = meta["T_b"], meta["tiles_total"]
    Tmax = max(T_b)
    col = np.concatenate([[0], np.cumsum(T_b)]).astype(int)
    C = meta["C"]
    kap = (1.0 - asig2, 1.0)  # skip-add scale on stored input, per layer

    nc = bacc.Bacc("TRN2", target_bir_lowering=False, debug=False,
                   num_devices=CORES)

    # --- I/O -------------------------------------------------------------
    xT_in = nc.dram_tensor("xT", [C, NPC], F16, kind="ExternalInput").ap()
    ekv_in = nc.dram_tensor("ekv", [128, tiles_total], I32,
                            kind="ExternalInput").ap()
    eslot_in = nc.dram_tensor("eslot", [128, tiles_total], BF16,
                              kind="ExternalInput").ap()
    s2_in = nc.dram_tensor("s2", [128, tiles_total * 128], BF16,
                           kind="ExternalInput").ap()
    wspec = {}
    for l in (1, 2):
        for n in ("Wq", "Wk", "Wv", "Wo"):
            wspec[f"{n}{l}"] = ([128, C], F16)
        for n in ("bq", "bk", "bv"):
            wspec[f"{n}{l}"] = ([128, C], F32)
        wspec[f"boa{l}"] = ([128, 1], F32)
    wspec["w12"] = ([128, 2], F16)
    w_in = {n: nc.dram_tensor(n, s, d, kind="ExternalInput").ap()
            for n, (s, d) in wspec.items()}
    uv_out = nc.dram_tensor("uv_out", [2, NPC], F32, kind="ExternalOutput").ap()

    # --- DRAM scratch ----------------------------------------------------
    kv_shard = [nc.dram_tensor(f"kv_shard{l}", [NPC, 2 * C], F16,
                               kind="Internal").ap() for l in (0, 1)]
    kv_full = [nc.dram_tensor(f"kv_full{l}", [NPAD, 2 * C], F16,
                              kind="Internal").ap() for l in (0, 1)]

    with tile.TileContext(nc) as tc, ExitStack() as ctx:
        cpool = ctx.enter_context(tc.tile_pool(name="const", bufs=1))
        sb = ctx.enter_context(tc.tile_pool(name="sb", bufs=2))
        psum = ctx.enter_context(tc.tile_pool(name="ps", bufs=2, space="PSUM"))

        # --- constants into SBUF ----------------------------------------
        W = {}
        for n, (s, d) in wspec.items():
            W[n] = cpool.tile(s, d, tag=f"w_{n}", name=f"wt_{n}")
            nc.sync.dma_start(W[n][:], w_in[n][:])
        ekv_sb = cpool.tile([128, tiles_total], I32, tag="ekv")
        nc.sync.dma_start(ekv_sb[:], ekv_in[:])
        eslot_sb = cpool.tile([128, tiles_total], BF16, tag="eslot")
        nc.sync.dma_start(eslot_sb[:], eslot_in[:])

        ident = cpool.tile([128, 128], F32, tag="ident")
        make_identity(nc, ident[:])
        iota_i = cpool.tile([128, Tmax * 128], I32, tag="iota_i")
        nc.gpsimd.iota(iota_i[:], pattern=[[0, Tmax], [1, 128]], base=0,
                       channel_multiplier=0)
        iota_bf = cpool.tile([128, Tmax * 128], BF16, tag="iota_bf")
        nc.vector.tensor_copy(iota_bf[:], iota_i[:])

        xT_all = cpool.tile([128, NPC], F16, tag="xT_all")
        nc.sync.dma_start(xT_all[:], xT_in[:])
        h1T_all = cpool.tile([128, NPC], F16, tag="h1T")
        aggn_all = [cpool.tile([128, NPC], BF16, tag=f"aggn{l}", name=f"aggn{l}")
                    for l in (0, 1)]
        uv_all = cpool.tile([2, NPC], F32, tag="uv")
        q_all = [cpool.tile([128, NPC], F16, tag=f"q_all{l}", name=f"q_all{l}")
                 for l in (0, 1)]

        def layer(li, srcT_all):
            l = li + 1
            kvs_d, kvf = kv_shard[li], kv_full[li]
            qa = q_all[li]
            # ---- projections for own shard ----
            for b in range(BPC):
                sl = slice(b * 128, (b + 1) * 128)
                lhs = srcT_all[:, sl]
                q_ps = psum.tile([128, C], F32, tag="mm128")
                nc.tensor.matmul(out=q_ps[:], lhsT=lhs, rhs=W[f"Wq{l}"][:],
                                 start=True, stop=True)
                nc.vector.tensor_tensor(out=qa[:, sl], in0=q_ps[:],
                                        in1=W[f"bq{l}"][:], op=OP.add)
                kvs = sb.tile([128, 2 * C], F16, tag="kvs")
                k_ps = psum.tile([128, C], F32, tag="mm128")
                nc.tensor.matmul(out=k_ps[:], lhsT=lhs, rhs=W[f"Wk{l}"][:],
                                 start=True, stop=True)
                nc.vector.tensor_tensor(out=kvs[:, 0:C], in0=k_ps[:],
                                        in1=W[f"bk{l}"][:], op=OP.add)
                v_ps = psum.tile([128, C], F32, tag="mm128")
                nc.tensor.matmul(out=v_ps[:], lhsT=lhs, rhs=W[f"Wv{l}"][:],
                                 start=True, stop=True)
                nc.vector.tensor_tensor(out=kvs[:, C:2 * C], in0=v_ps[:],
                                        in1=W[f"bv{l}"][:], op=OP.add)
                nc.sync.dma_start(kvs_d[sl, :], kvs[:])
            # ---- exchange k/v ----
            nc.gpsimd.collective_compute(
                "AllGather", OP.bypass,
                replica_groups=[list(range(CORES))],
                ins=[kvs_d[:]], outs=[kvf[:]])
            # ---- edge phase ----
            for b in range(BPC):
                T = T_b[b]
                c0 = int(col[b])
                kvg = sb.tile([128, Tmax * 2 * C], F16, tag="kvg")
                for t in range(T):
                    nc.gpsimd.indirect_dma_start(
                        out=kvg[:, t * 256:(t + 1) * 256], out_offset=None,
                        in_=kvf,
                        in_offset=bass.IndirectOffsetOnAxis(
                            ap=ekv_sb[:, c0 + t:c0 + t + 1], axis=0))
                S2 = sb.tile([128, Tmax * 128], BF16, tag="S2")
                nc.sync.dma_start(
                    S2[:, :T * 128],
                    s2_in[:, c0 * 128:(c0 + T) * 128])
                qg = sb.tile([128, Tmax * C], F16, tag="qg")
                for t in range(T):
                    qg_ps = psum.tile([128, C], F32, tag="qg")
                    nc.tensor.matmul(out=qg_ps[:],
                                     lhsT=S2[:, t * 128:(t + 1) * 128],
                                     rhs=qa[:, b * 128:(b + 1) * 128],
                                     start=True, stop=True)
                    nc.vector.tensor_copy(qg[:, t * 128:(t + 1) * 128],
                                          qg_ps[:])
                S = sb.tile([128, Tmax * 128], BF16, tag="S")
                nc.vector.tensor_tensor(
                    out=_apn(S[:], [[128, T], [1, 128]]),
                    in0=_apn(iota_bf[:], [[128, T], [1, 128]]),
                    in1=_expand_last(eslot_sb[:, c0:c0 + T], 128),
                    op=OP.is_equal)
                prod = sb.tile([128, Tmax * C], F16, tag="prod")
                nc.vector.tensor_tensor(
                    out=_apn(prod[:], [[128, T], [1, 128]]),
                    in0=_apn(kvg[:], [[256, T], [1, 128]]),
                    in1=_apn(qg[:], [[128, T], [1, 128]]), op=OP.mult)
                alpha = sb.tile([128, Tmax * 4], F32, tag="alpha")
                nc.vector.tensor_reduce(
                    out=alpha[:, :T * 4],
                    in_=_apn(prod[:], [[32, T * 4], [1, 32]]),
                    axis=mybir.AxisListType.X, op=OP.add)
                ex = sb.tile([128, Tmax * 4], BF16, tag="ex")
                nc.scalar.activation(ex[:, :T * 4], alpha[:, :T * 4], AF.Exp)
                evex = sb.tile([128, Tmax * 132], BF16, tag="evex")
                nc.scalar.activation(_apn(evex[:], [[132, T], [1, 4]]),
                                     alpha[:, :T * 4], AF.Exp)
                nc.vector.tensor_tensor(
                    out=_apn(evex[:], [[132, T], [32, 4], [1, 32]], off=4),
                    in0=_apn(kvg[:], [[256, T], [32, 4], [1, 32]], off=128),
                    in1=_apn(ex[:], [[4, T], [1, 4], [0, 32]]), op=OP.mult)
                agg_ps = psum.tile([128, 132], F32, tag="mm132")
                for t in range(T):
                    nc.tensor.matmul(out=agg_ps[:],
                                     lhsT=S[:, t * 128:(t + 1) * 128],
                                     rhs=evex[:, t * 132:(t + 1) * 132],
                                     start=(t == 0), stop=(t == T - 1))
                den = sb.tile([128, 4], F32, tag="den")
                nc.vector.tensor_scalar_add(den[:], agg_ps[:, 0:4], EPS)
                rd = sb.tile([128, 4], F32, tag="rd")
                nc.vector.reciprocal(rd[:], den[:])
                nc.vector.tensor_tensor(
                    out=_apn(aggn_all[li][:], [[32, 4], [1, 32]], off=b * 128),
                    in0=_apn(agg_ps[:], [[32, 4], [1, 32]], off=4),
                    in1=_expand_last(rd[:], 32), op=OP.mult)
            # ---- pass 2: gelu, output projection, skip (+ decode) ----
            for b in range(BPC):
                sl = slice(b * 128, (b + 1) * 128)
                g = sb.tile([128, C], F32, tag="g")
                nc.scalar.activation(g[:], aggn_all[li][:, sl], AF.Gelu)
                gT_ps = psum.tile([128, C], F32, tag="tr")
                nc.tensor.transpose(out=gT_ps[:], in_=g[:], identity=ident[:])
                gT = sb.tile([128, C], BF16, tag="gTs")
                nc.vector.tensor_copy(gT[:], gT_ps[:])
                hm_ps = psum.tile([128, C], F32, tag="mm128")
                nc.tensor.matmul(out=hm_ps[:], lhsT=W[f"Wo{l}"][:], rhs=gT[:],
                                 start=True, stop=True)
                if l == 1:
                    zt_ap = h1T_all[:, sl]
                else:
                    zt = sb.tile([128, C], F16, tag="zt")
                    zt_ap = zt[:]
                nc.vector.scalar_tensor_tensor(
                    out=zt_ap, in0=srcT_all[:, sl], scalar=kap[li],
                    in1=hm_ps[:], op0=OP.mult, op1=OP.add)
                nc.vector.tensor_tensor(
                    out=zt_ap, in0=zt_ap,
                    in1=_apn(W[f"boa{l}"][:], [[0, 128]]), op=OP.add)
                if l == 2:
                    uv_ps = psum.tile([128, C], F32, tag="mm128")
                    nc.tensor.matmul(out=uv_ps[0:2, :], lhsT=W["w12"][:],
                                     rhs=zt_ap, start=True, stop=True)
                    nc.vector.tensor_copy(uv_all[:, sl], uv_ps[0:2, :])

        layer(0, xT_all)
        layer(1, h1T_all)
        nc.sync.dma_start(uv_out, uv_all[:])

    nc.compile()
    return nc


_CACHE = {}


def _get_program(meta, asig1, asig2, blp):
    key = (meta["N"], meta["E"], meta["T_b"], asig1, asig2)
    if key not in _CACHE:
        _CACHE[key] = _build_program(meta, asig1, asig2)
    return _CACHE[key]


def make_in_maps(inputs):
    inputs = {k: np.asarray(v) for k, v in inputs.items()}
    meta, arrays = _host_prep(np.asarray(inputs["x"], np.float32),
                              inputs["edge_index"])
    w = _prep_weights(inputs)
    N, C, NPC = meta["N"], meta["C"], meta["NPC"]
    xpad = np.zeros((meta["NPAD"], C), dtype=np.float64)
    xpad[:N] = np.asarray(inputs["x"], np.float64)
    xT_full = np.ascontiguousarray((xpad * w["xscale"]).T).astype(np.float16)
    in_maps = []
    for c in range(CORES):
        m = dict(xT=np.ascontiguousarray(xT_full[:, c * NPC:(c + 1) * NPC]),
                 ekv=arrays["ekv"][c], eslot=arrays["eslot"][c],
                 s2=arrays["s2"][c])
        for n in w["names"]:
            m[n] = w[n]
        in_maps.append(m)
    return meta, w, in_maps


def assemble(meta, results, inputs, blp):
    u = np.concatenate([results[c]["uv_out"] for c in range(CORES)], axis=1)
    u1, u2 = u[0], u[1]
    pe, ne = inputs["pos_edge_index"], inputs["neg_edge_index"]
    pos = u1[pe[0]] + u2[pe[1]] + np.float32(blp)
    neg = u1[ne[0]] + u2[ne[1]] + np.float32(blp)
    return pos.astype(np.float32), neg.astype(np.float32)


def kernel(**inputs):
    meta, w, in_maps = make_in_maps(inputs)
    nc = _get_program(meta, w["asig1"], w["asig2"], w["blp"])
    res = bass_utils.run_bass_kernel_spmd(nc, in_maps,
                                          core_ids=list(range(CORES)))
    return assemble(meta, res.results, inputs, w["blp"])
